# revision 2
# baseline (speedup 1.0000x reference)
"""MDRNN 2D-grid recurrence kernel for 8 Trainium2 NeuronCores.

h[i,j] = tanh(x[i,j] @ w + h[i-1,j]*u0 + h[i,j-1]*u1 + bias)

Strategy:
  - Data-parallel over batch: B=16 -> 2 batch elements per core, run as
    two independent anti-diagonal wavefront chains (b-major layout) so
    their dependency chains interleave on the engines.
  - Host pre-transposes x into diag-ordered [SIN+1, b, cells] layout
    (ones row appended so the GEMM also adds the bias).
  - GEMM (w stationary) runs ahead in 512-column PSUM chunks; the
    wavefront reads a' straight out of PSUM.
  - Per anti-diagonal d and batch half: two fused scalar_tensor_tensor
    ops on DVE (t1 = h_left*u1 + a'; z = h_up*u0 + t1), one ACT tanh
    into an i-aligned zero-margined ring buffer, DMA out diag-packed.
  - Host inverse-permutes the diag-packed output back to (i,j,b,o).
"""

import numpy as np

D1, D2, B, SIN, SOUT = 128, 128, 16, 64, 128
NCORES = 8
BLOC = B // NCORES  # 2
NCELLS = D1 * D2  # 16384 = cols per batch half
NCOLS = NCELLS * BLOC  # 32768
ND = D1 + D2 - 1  # 255
CHUNK = 512  # GEMM chunk (one PSUM bank of fp32)
NCHUNKS_H = NCELLS // CHUNK  # 32 per batch half


def _diag_order():
    I, J, bases = [], [], [0]
    for d in range(ND):
        i0 = max(0, d - (D2 - 1))
        i1 = min(D1 - 1, d)
        for i in range(i0, i1 + 1):
            I.append(i)
            J.append(d - i)
        bases.append(len(I))
    return np.array(I), np.array(J), np.array(bases)


_CACHE = {}


def _build_program():
    if "nc" in _CACHE:
        return _CACHE["nc"]
    import concourse.mybir as mybir
    from concourse import bacc
    import concourse.bass as bass
    from concourse.tile import TileContext

    f32 = mybir.dt.float32
    mult = mybir.AluOpType.mult
    add = mybir.AluOpType.add
    Tanh = mybir.ActivationFunctionType.Tanh

    _, _, bases = _diag_order()

    nc = bacc.Bacc(None, target_bir_lowering=False)
    xa = nc.dram_tensor("xa", (SIN + 1, NCOLS), f32, kind="ExternalInput")
    wb = nc.dram_tensor("wb", (SIN + 1, SOUT), f32, kind="ExternalInput")
    uu = nc.dram_tensor("uu", (SOUT, 2), f32, kind="ExternalInput")
    ho = nc.dram_tensor("ho", (SOUT, NCOLS), f32, kind="ExternalOutput")

    K = 4  # ring depth per batch half
    RW = D1 + 1  # 129 slots: i = -1..127

    with TileContext(nc) as tc:
        with (
            tc.tile_pool(name="const", bufs=1) as constp,
            tc.tile_pool(name="xbig", bufs=1) as xbigp,
            tc.tile_pool(name="ring", bufs=1) as ringp,
            tc.tile_pool(name="scratch", bufs=4) as scrp,
            tc.tile_pool(name="psum", bufs=8, space=bass.MemorySpace.PSUM) as psump,
        ):
            wb_sb = constp.tile([SIN + 1, SOUT], f32, tag="wb")
            nc.sync.dma_start(wb_sb[:], wb[:])
            u_sb = constp.tile([SOUT, 2], f32, tag="uu")
            nc.sync.dma_start(u_sb[:], uu[:])
            u0 = u_sb[:, 0:1]
            u1 = u_sb[:, 1:2]

            xa_sb = xbigp.tile([SIN + 1, NCOLS], f32, tag="xa")
            csz = 2048
            for k in range(NCOLS // csz):
                nc.sync.dma_start(
                    xa_sb[:, k * csz : (k + 1) * csz],
                    xa[:, k * csz : (k + 1) * csz],
                )

            rings = []
            for hb in range(BLOC):
                rr = []
                for m in range(K):
                    t = ringp.tile([SOUT, RW], f32, tag=f"ring{hb}_{m}")
                    nc.vector.memset(t[:], 0.0)
                    rr.append(t)
                rings.append(rr)

            # GEMM chunks, emitted on demand with one-chunk lookahead.
            # chunk index g in [0, 64): covers xa cols [g*512, g*512+512)
            psch = [None] * (2 * NCHUNKS_H)

            def emit_chunk(g):
                if psch[g] is not None:
                    return
                ps = psump.tile([SOUT, CHUNK], f32, tag="ps")
                nc.tensor.matmul(
                    out=ps[:],
                    lhsT=wb_sb[:],
                    rhs=xa_sb[:, g * CHUNK : (g + 1) * CHUNK],
                    start=True,
                    stop=True,
                )
                psch[g] = ps

            # prime the pipeline: first two chunks of each half
            for hb in range(BLOC):
                emit_chunk(hb * NCHUNKS_H)
                emit_chunk(hb * NCHUNKS_H + 1)

            for d in range(ND):
                i0 = max(0, d - (D2 - 1))
                i1 = min(D1 - 1, d)
                C = i1 - i0 + 1
                base = int(bases[d])
                for hb in range(BLOC):
                    goff = hb * NCHUNKS_H
                    prev = rings[hb][(d - 1) % K]
                    cur = rings[hb][d % K]
                    # prefetch: ensure chunks covering this diag + next
                    glast = (base + C - 1) // CHUNK
                    for g in range(base // CHUNK, min(glast + 2, NCHUNKS_H)):
                        emit_chunk(goff + g)

                    t1 = scrp.tile([SOUT, D1], f32, tag=f"t1_{hb}")
                    # t1 = h_left * u1 + a', split at PSUM chunk boundaries
                    pos = 0
                    while pos < C:
                        col = base + pos
                        g = col // CHUNK
                        take = min(C - pos, (g + 1) * CHUNK - col)
                        nc.vector.scalar_tensor_tensor(
                            out=t1[:, pos : pos + take],
                            in0=prev[:, i0 + 1 + pos : i0 + 1 + pos + take],
                            scalar=u1,
                            in1=psch[goff + g][:, col - g * CHUNK : col - g * CHUNK + take],
                            op0=mult,
                            op1=add,
                        )
                        pos += take
                    t2 = scrp.tile([SOUT, D1], f32, tag=f"t2_{hb}")
                    # z = h_up * u0 + t1
                    nc.vector.scalar_tensor_tensor(
                        out=t2[:, :C],
                        in0=prev[:, i0 : i1 + 1],
                        scalar=u0,
                        in1=t1[:, :C],
                        op0=mult,
                        op1=add,
                    )
                    nc.scalar.activation(
                        out=cur[:, i0 + 1 : i1 + 2],
                        in_=t2[:, :C],
                        func=Tanh,
                    )
                    nc.sync.dma_start(
                        ho[:, hb * NCELLS + base : hb * NCELLS + base + C],
                        cur[:, i0 + 1 : i1 + 2],
                    )

    nc.compile()
    _CACHE["nc"] = nc
    return nc


def _prep_inputs(x, w, u, bias):
    I, J, _ = _diag_order()
    xa_cells = np.ascontiguousarray(x[I, J])  # (16384, B, SIN)
    wbm = np.concatenate([w, bias[None, :]], axis=0).astype(np.float32)  # (65,128)
    um = np.ascontiguousarray(u.T).astype(np.float32)  # (128, 2): col0=u0, col1=u1
    in_maps = []
    for c in range(NCORES):
        xc = xa_cells[:, c * BLOC : (c + 1) * BLOC, :]  # (16384, 2, 64)
        xc = xc.transpose(2, 1, 0).reshape(SIN, NCOLS)  # (64, 2*16384) b-major
        xc = np.concatenate([xc, np.ones((1, NCOLS), np.float32)], axis=0)
        in_maps.append(
            {"xa": np.ascontiguousarray(xc), "wb": wbm, "uu": um}
        )
    return in_maps


def _assemble(results):
    I, J, _ = _diag_order()
    out = np.zeros((D1, D2, B, SOUT), np.float32)
    for c in range(NCORES):
        hoc = results[c]["ho"]  # (128, 32768) b-major
        h_core = hoc.reshape(SOUT, BLOC, NCELLS).transpose(2, 1, 0)
        out[I, J, c * BLOC : (c + 1) * BLOC, :] = h_core
    return out


def kernel(x, w, u, bias, _trace=False):
    from concourse.bass_utils import run_bass_kernel_spmd

    x = np.asarray(x, dtype=np.float32)
    w = np.asarray(w, dtype=np.float32)
    u = np.asarray(u, dtype=np.float32)
    bias = np.asarray(bias, dtype=np.float32)

    nc = _build_program()
    in_maps = _prep_inputs(x, w, u, bias)
    res = run_bass_kernel_spmd(
        nc, in_maps, core_ids=list(range(NCORES)), trace=_trace
    )
    _CACHE["last_result"] = res
    return _assemble(res.results)


# revision 4
# speedup vs baseline: 1.3195x; 1.3195x over previous
"""MDRNN 2D-grid recurrence kernel for 8 Trainium2 NeuronCores.

h[i,j] = tanh(x[i,j] @ w + h[i-1,j]*u0 + h[i,j-1]*u1 + bias)

Strategy:
  - Data-parallel over batch: B=16 -> 2 batch elements per core.
  - Host pre-transposes x into anti-diagonal-ordered [SIN+1, cells*b]
    layout (ones row appended so the GEMM adds the bias in PSUM).
  - GEMM (w stationary) runs ahead of the wavefront in PSUM chunks that
    are aligned to whole diagonals (<=512 cols); the wavefront reads a'
    straight out of PSUM.
  - Per anti-diagonal d: two fused scalar_tensor_tensor ops on DVE
    (t1 = h_left*u1 + a'; z = h_up*u0 + t1) and one ACT tanh that
    writes a packed staging buffer with one zero "gap" pair between
    diagonals -- the gaps provide the recurrence boundary zeros, so
    consecutive diagonals read each other with plain contiguous slices.
  - Output DMA is batched: one DMA per 2048-col staging segment.
  - Host inverse-permutes the gap-padded diag-ordered output to
    (i,j,b,o).
"""

import numpy as np

D1, D2, B, SIN, SOUT = 128, 128, 16, 64, 128
NCORES = 8
BLOC = B // NCORES  # 2
NCELLS = D1 * D2
NCOLS = NCELLS * BLOC  # 32768
ND = D1 + D2 - 1  # 255
GAP = BLOC  # one zero cell-pair between diagonals
NCOLS_G = NCOLS + GAP * ND + GAP  # 33280: staging/out cols incl gaps
SEG = 2048  # output DMA segment (cols)
NSEG = (NCOLS_G + SEG - 1) // SEG  # 17 (last partial)


def _diag_geom():
    """Per-diag (i0, C); packed bases; gapped bases."""
    geo, bases, gbases = [], [0], [GAP]
    for d in range(ND):
        i0 = max(0, d - (D2 - 1))
        i1 = min(D1 - 1, d)
        C = i1 - i0 + 1
        geo.append((i0, C))
        bases.append(bases[-1] + C)
        gbases.append(gbases[-1] + C * BLOC + GAP)
    return geo, bases, gbases


def _diag_order():
    I, J = [], []
    for d in range(ND):
        for i in range(max(0, d - (D2 - 1)), min(D1 - 1, d) + 1):
            I.append(i)
            J.append(d - i)
    return np.array(I), np.array(J)


_CACHE = {}


def _build_program():
    if "nc" in _CACHE:
        return _CACHE["nc"]
    import concourse.mybir as mybir
    from concourse import bacc
    import concourse.bass as bass
    from concourse.tile import TileContext

    f32 = mybir.dt.float32
    mult = mybir.AluOpType.mult
    add = mybir.AluOpType.add
    Tanh = mybir.ActivationFunctionType.Tanh

    geo, bases, gbases = _diag_geom()

    # GEMM chunks = greedy groups of whole diagonals, <=512 cols each.
    chunks = []  # (start_diag, end_diag, col0, ncols)
    d0 = 0
    while d0 < ND:
        col0 = bases[d0] * BLOC
        d1 = d0
        while d1 + 1 < ND and (bases[d1 + 2] * BLOC - col0) <= 512:
            d1 += 1
        chunks.append((d0, d1, col0, bases[d1 + 1] * BLOC - col0))
        d0 = d1 + 1
    chunk_of_diag = {}
    for ci, (a, b, _, _) in enumerate(chunks):
        for d in range(a, b + 1):
            chunk_of_diag[d] = ci

    nc = bacc.Bacc(None, target_bir_lowering=False)
    xa = nc.dram_tensor("xa", (SIN + 1, NCOLS), f32, kind="ExternalInput")
    wb = nc.dram_tensor("wb", (SIN + 1, SOUT), f32, kind="ExternalInput")
    uu = nc.dram_tensor("uu", (SOUT, 2), f32, kind="ExternalInput")
    ho = nc.dram_tensor("ho", (SOUT, NCOLS_G), f32, kind="ExternalOutput")

    XCH = 2048  # xa streaming chunk (cols)

    with TileContext(nc) as tc:
        with (
            tc.tile_pool(name="const", bufs=1) as constp,
            tc.tile_pool(name="xring", bufs=4) as xringp,
            tc.tile_pool(name="stage", bufs=1) as stagep,
            tc.tile_pool(name="scratch", bufs=4) as scrp,
            tc.tile_pool(name="psum", bufs=8, space=bass.MemorySpace.PSUM) as psump,
        ):
            wb_sb = constp.tile([SIN + 1, SOUT], f32, tag="wb")
            nc.sync.dma_start(wb_sb[:], wb[:])
            u_sb = constp.tile([SOUT, 2], f32, tag="uu")
            nc.sync.dma_start(u_sb[:], uu[:])
            u0 = u_sb[:, 0:1]
            u1 = u_sb[:, 1:2]

            stage = stagep.tile([SOUT, NCOLS_G], f32, tag="stage")
            # zero-fill staging (gaps must read as 0); split so early
            # segments are ready fast. GpSimd keeps it off DVE/ACT.
            for s in range(NSEG):
                lo = s * SEG
                hi = min(lo + SEG, NCOLS_G)
                nc.gpsimd.memset(stage[:, lo:hi], 0.0)

            # xa streaming ring
            xtiles = [None] * (NCOLS // XCH)

            def load_x(k):
                if xtiles[k] is None:
                    t = xringp.tile([SIN + 1, XCH], f32, tag="xa")
                    nc.sync.dma_start(t[:], xa[:, k * XCH : (k + 1) * XCH])
                    xtiles[k] = t

            # GEMM chunk -> PSUM (diag-aligned, may span two xa tiles)
            pstile = [None] * len(chunks)

            def emit_chunk(ci):
                if pstile[ci] is not None:
                    return
                _, _, col0, ncols = chunks[ci]
                ps = psump.tile([SOUT, 512], f32, tag="ps")
                pos = 0
                while pos < ncols:
                    k = (col0 + pos) // XCH
                    load_x(k)
                    if k + 1 < len(xtiles):
                        load_x(k + 1)
                    take = min(ncols - pos, (k + 1) * XCH - (col0 + pos))
                    off = col0 + pos - k * XCH
                    nc.tensor.matmul(
                        out=ps[:, pos : pos + take],
                        lhsT=wb_sb[:],
                        rhs=xtiles[k][:, off : off + take],
                        start=True,
                        stop=True,
                    )
                    pos += take
                pstile[ci] = ps

            emit_chunk(0)
            emit_chunk(1)

            seg_done = 0
            for d in range(ND):
                i0, C = geo[d]
                n = C * BLOC
                gb = gbases[d]
                pgb = gbases[d - 1] if d > 0 else 0  # d=0: zeros at [0,GAP)
                ci = chunk_of_diag[d]
                emit_chunk(ci)
                if ci + 1 < len(chunks):
                    emit_chunk(ci + 1)
                ps = pstile[ci]
                poff = bases[d] * BLOC - chunks[ci][2]

                if d > 0 and geo[d - 1][0] < i0:
                    # shrinking phase: prev diag starts one cell lower
                    hl = stage[:, pgb + GAP : pgb + GAP + n]
                    hu = stage[:, pgb : pgb + n]
                else:
                    # growing: top boundary = gap before prev, bottom =
                    # gap after prev (both zero)
                    hus = pgb - GAP if d > 0 else 0
                    hl = stage[:, pgb : pgb + n]
                    hu = stage[:, hus : hus + n]

                t1 = scrp.tile([SOUT, 256], f32, tag="t1")
                nc.vector.scalar_tensor_tensor(
                    out=t1[:, :n],
                    in0=hl,
                    scalar=u1,
                    in1=ps[:, poff : poff + n],
                    op0=mult,
                    op1=add,
                )
                t2 = scrp.tile([SOUT, 256], f32, tag="t2")
                nc.vector.scalar_tensor_tensor(
                    out=t2[:, :n],
                    in0=hu,
                    scalar=u0,
                    in1=t1[:, :n],
                    op0=mult,
                    op1=add,
                )
                nc.scalar.activation(
                    out=stage[:, gb : gb + n],
                    in_=t2[:, :n],
                    func=Tanh,
                )
                # flush finished staging segments
                while (seg_done + 1) * SEG <= gb:
                    lo = seg_done * SEG
                    nc.sync.dma_start(ho[:, lo : lo + SEG], stage[:, lo : lo + SEG])
                    seg_done += 1
            while seg_done * SEG < NCOLS_G:
                lo = seg_done * SEG
                hi = min(lo + SEG, NCOLS_G)
                nc.sync.dma_start(ho[:, lo:hi], stage[:, lo:hi])
                seg_done += 1

    nc.compile()
    _CACHE["nc"] = nc
    return nc


def _prep_inputs(x, w, u, bias):
    I, J = _diag_order()
    xa_cells = np.ascontiguousarray(x[I, J])  # (16384, B, SIN)
    wbm = np.concatenate([w, bias[None, :]], axis=0).astype(np.float32)
    um = np.ascontiguousarray(u.T).astype(np.float32)  # (128,2): u0,u1 cols
    in_maps = []
    for c in range(NCORES):
        xc = xa_cells[:, c * BLOC : (c + 1) * BLOC, :]  # (16384, 2, 64)
        xc = xc.transpose(2, 0, 1).reshape(SIN, NCOLS)  # cell-major
        xc = np.concatenate([xc, np.ones((1, NCOLS), np.float32)], axis=0)
        in_maps.append({"xa": np.ascontiguousarray(xc), "wb": wbm, "uu": um})
    return in_maps


def _assemble(results):
    I, J = _diag_order()
    geo, bases, gbases = _diag_geom()
    valid = np.zeros(NCOLS, np.int64)
    for d in range(ND):
        n = geo[d][1] * BLOC
        valid[bases[d] * BLOC : bases[d] * BLOC + n] = gbases[d] + np.arange(n)
    out = np.zeros((D1, D2, B, SOUT), np.float32)
    for c in range(NCORES):
        hoc = results[c]["ho"][:, valid]  # (128, 32768) packed
        h_core = hoc.reshape(SOUT, NCELLS, BLOC).transpose(1, 2, 0)
        out[I, J, c * BLOC : (c + 1) * BLOC, :] = h_core
    return out


def kernel(x, w, u, bias, _trace=False):
    from concourse.bass_utils import run_bass_kernel_spmd

    x = np.asarray(x, dtype=np.float32)
    w = np.asarray(w, dtype=np.float32)
    u = np.asarray(u, dtype=np.float32)
    bias = np.asarray(bias, dtype=np.float32)

    nc = _build_program()
    in_maps = _prep_inputs(x, w, u, bias)
    res = run_bass_kernel_spmd(
        nc, in_maps, core_ids=list(range(NCORES)), trace=_trace
    )
    _CACHE["last_result"] = res
    return _assemble(res.results)
